# revision 32
# baseline (speedup 1.0000x reference)
"""Multi-head attention (B=8, S=1024, D=1024, H=16, dk=dv=64) on 8 TRN2 cores.

Sharding: data-parallel over batch — core b computes batch element b end to
end; no collectives. Host-side prep transposes activations/weights into the
layouts TensorE needs (contraction dim on partitions); all matmuls run on
device in bf16 (fp32 psum accumulate).

Per-core dataflow (everything "T" = [feature, seq] layout):
  v projection first (own 8-bank psum pool; inputs stream d-ordered in
  64-row chunks split across the sync+scalar DMA issue queues so the first
  contraction steps land early).
  Pipelined head-pair loop (a = 0..7, heads 2a/2a+1 on PE row strips):
    qkproj(a): weight blocks stream just-in-time; q/k tiles rotate (bufs=3)
    scores(a-1): per (s2,c) one [128,1024] fp32 psum tile spanning 2 banks;
      the two heads' K=64 matmuls run concurrently via tile_position row
      strips; ONE fused exp [128,1024] on ScalarE -> bf16
    pv(a-2): both heads' PV matmuls run concurrently on column strips
      0-63/64-127 (M=64 each, full 128-row contraction); softmax
      denominators come from four col-tiled ones-matmuls (32-col strips,
      ~free under the PV pair); denominators copied to a partition-0 row,
      reciprocal_approx_fast in place (SBUF only - the PSUM-input path of
      the custom DVE op returns garbage), cast to bf16
    norm(a-3): emitted at the END of the iteration so the gpsimd
      partition_broadcast + DVE multiply never sit in the DVE FIFO ahead of
      the next iteration's projection copies (cross-engine convoy)
  fc split ct0-3 (iters 6/7, bf16 partials) / ct4-6 (early drain, in-place
  adds) / ct7 (after the last norm; evictions alternate DVE add and
  ScalarE-copy+GpSimd-add; psum from a shared 4-slot rotation).
  Drain norms use PE rank-1 col-tiled broadcasts (ones.T @ recip-row,
  both heads into one psum tile) + one full-height multiply per chunk
  instead of the 1.7us gpsimd broadcast - the PE is idle there. The
  drain's long-lived denominator bank parks in the then-idle scores psum
  slots. q/k weights arrive host-blocked so each head-pair's column block
  is one contiguous dma (2 issues/iteration instead of 16).
"""

import numpy as np

import concourse.bacc as bacc
import concourse.mybir as mybir
import concourse.tile as tile
from concourse.bass_utils import run_bass_kernel_spmd

S = 1024
D = 1024
H = 16
DK = 64
P = 128
NT = S // P          # 8 seq/feature tiles
NCH = 2              # 512-wide free-dim chunks
CH = S // NCH        # 512
F32 = mybir.dt.float32
BF16 = mybir.dt.bfloat16
EXP = mybir.ActivationFunctionType.Exp

_CACHE = {}


def _build():
    nc = bacc.Bacc("TRN2", target_bir_lowering=False, debug=False)
    xqt = nc.dram_tensor("xqt", [D, S], BF16, kind="ExternalInput").ap()
    xkt = nc.dram_tensor("xkt", [D, S], BF16, kind="ExternalInput").ap()
    xvt = nc.dram_tensor("xvt", [D, S], BF16, kind="ExternalInput").ap()
    # q/k weights arrive host-blocked: wqb[a, p, d*128+m] = WQ^T[d*128+p,
    # a*128+m], so each head-pair's column block is one contiguous 2D dma
    wqb = nc.dram_tensor("wqb", [NT, P, D], BF16, kind="ExternalInput").ap()
    wkb = nc.dram_tensor("wkb", [NT, P, D], BF16, kind="ExternalInput").ap()
    wvt = nc.dram_tensor("wvt", [D, D], BF16, kind="ExternalInput").ap()
    wft = nc.dram_tensor("wft", [D, D], BF16, kind="ExternalInput").ap()
    out = nc.dram_tensor("out", [S, D], F32, kind="ExternalOutput").ap()

    from contextlib import ExitStack

    with tile.TileContext(nc) as tc:
        with (
            tc.tile_pool(name="persist", bufs=1) as pp,
        ):
            # v natural layout [seq, features]
            vv = [pp.tile([P, D], BF16, tag=f"v{t}", name=f"v{t}")
                  for t in range(NT)]
            ctxT = [pp.tile([P, S], BF16, tag=f"c{t}", name=f"c{t}")
                    for t in range(NT)]

            with ExitStack() as stk:
                ap_ = stk.enter_context(tc.tile_pool(name="attn", bufs=2))
                xtq = [ap_.tile([P, S], BF16, tag="xtq", name="xtq", bufs=8)
                       for _ in range(NT)]
                xtk = [ap_.tile([P, S], BF16, tag="xtk", name="xtk", bufs=8)
                       for _ in range(NT)]
                # fc weights for ct 0-3 get their own slots so the early fc
                # chunks can run while xtq is still live
                wf4 = [ap_.tile([P, S], BF16, tag="wf4", name="wf4", bufs=4)
                       for _ in range(4)]

                # ---- v projection first (attention needs all of v) ----
                with tc.tile_pool(name="vld", bufs=8) as vp, \
                     tc.tile_pool(name="vps", bufs=8, space="PSUM") as vpsp:
                    xts = [vp.tile([P, S], BF16, tag="xt", name="xt")
                           for _ in range(NT)]
                    ws = [vp.tile([P, D], BF16, tag="w", name="w")
                          for _ in range(NT)]
                    # v inputs chunked + d-ordered, split across BOTH
                    # hwdge issue queues (sync + scalar) so they land first;
                    # q/k follow, fc weights last (needed only at iter 6)
                    for t in range(NT):
                        if t < 2:
                            # quarter-chunk the first tiles, interleaving
                            # activations and weights across both issue
                            # queues: the first matmul needs xts[0]+ws[0]
                            # complete, so get all 8 quarters in flight at
                            # once
                            for q in range(4):
                                nc.sync.dma_start(
                                    out=xts[t][q * 32:(q + 1) * 32, :],
                                    in_=xvt[t * P + q * 32:
                                            t * P + (q + 1) * 32, :])
                                nc.scalar.dma_start(
                                    out=ws[t][q * 32:(q + 1) * 32, :],
                                    in_=wvt[t * P + q * 32:
                                            t * P + (q + 1) * 32, :])
                        else:
                            nc.sync.dma_start(
                                out=xts[t][0:64, :],
                                in_=xvt[t * P:t * P + 64, :])
                            nc.scalar.dma_start(
                                out=xts[t][64:128, :],
                                in_=xvt[t * P + 64:(t + 1) * P, :])
                            nc.sync.dma_start(
                                out=ws[t][0:64, :],
                                in_=wvt[t * P:t * P + 64, :])
                            nc.scalar.dma_start(
                                out=ws[t][64:128, :],
                                in_=wvt[t * P + 64:(t + 1) * P, :])
                    for t in range(NT):
                        enq = nc.sync if t % 2 == 0 else nc.scalar
                        enk = nc.scalar if t % 2 == 0 else nc.sync
                        enq.dma_start(out=xtq[t][:],
                                      in_=xqt[t * P:(t + 1) * P, :])
                        enk.dma_start(out=xtk[t][:],
                                      in_=xkt[t * P:(t + 1) * P, :])
                    for ct in range(4):
                        nc.scalar.dma_start(out=wf4[ct][:],
                                            in_=wft[ct * P:(ct + 1) * P, :])

                    for s2 in range(NT):
                        pss = [vpsp.tile([P, CH], F32, tag="vp", name="vp")
                               for _ in range(NCH)]
                        for d in range(NT):
                            for c in range(NCH):
                                nc.tensor.matmul(
                                    pss[c][:],
                                    lhsT=xts[d][:, s2 * P:(s2 + 1) * P],
                                    rhs=ws[d][:, c * CH:(c + 1) * CH],
                                    start=(d == 0),
                                    stop=(d == NT - 1),
                                )
                        for c in range(NCH):
                            nc.vector.tensor_copy(
                                vv[s2][:, c * CH:(c + 1) * CH], pss[c][:])

                # second SBUF pool for tags that only exist after the
                # v-load pool is gone (exp tiles, recip rows, fc partials) —
                # keeps the peak footprint under the SBUF limit
                ap2 = stk.enter_context(tc.tile_pool(name="attn2", bufs=2))
                # main psum pool for the attention loop: proj 2 + sc 4 + pv 2
                psp = stk.enter_context(
                    tc.tile_pool(name="psum", bufs=2, space="PSUM"))

                def qkproj(a):
                    # q/k head-pair tiles rotate (lifetime: this iteration's
                    # projection + next iteration's scores); the whole
                    # weight column-block is ONE contiguous dma
                    outs = []
                    for xts_, wsrc, tg in ((xtq, wqb, "qTr"), (xtk, wkb, "kTr")):
                        dst = pp.tile([P, S], BF16, tag=tg, name=tg, bufs=3)
                        wt8 = ap_.tile([P, D], BF16, tag="wqk", name="wqk",
                                       bufs=3)
                        nc.sync.dma_start(out=wt8[:], in_=wsrc[a])
                        pss = [psp.tile([P, CH], F32, tag="ps", name="proj",
                                        bufs=4)
                               for _ in range(NCH)]
                        for d in range(NT):
                            for c in range(NCH):
                                nc.tensor.matmul(
                                    pss[c][:],
                                    lhsT=wt8[:, d * P:(d + 1) * P],
                                    rhs=xts_[d][:, c * CH:(c + 1) * CH],
                                    start=(d == 0),
                                    stop=(d == NT - 1),
                                )
                        for c in range(NCH):
                            nc.vector.tensor_copy(
                                dst[:, c * CH:(c + 1) * CH], pss[c][:])
                        outs.append(dst)
                    return outs

                def scores(qk):
                    qTa, kTa = qk
                    # per (s2, c): one fp32 psum tile [128, 1024] spanning 2
                    # banks; the two heads' K=64 matmuls (N=512 each, row
                    # strips 0-63 / 64-127) run concurrently and each fill
                    # one bank; a single fused exp [128, 1024] reads both.
                    exps = []
                    for s2 in range(NT):
                        scs = [psp.tile([P, S], F32, tag="sc", name="sc")
                               for _ in range(NCH)]
                        for c in range(NCH):
                            for g in range(2):
                                nc.tensor.matmul(
                                    scs[c][:, g * CH:(g + 1) * CH],
                                    lhsT=kTa[g * DK:(g + 1) * DK,
                                             s2 * P:(s2 + 1) * P],
                                    rhs=qTa[g * DK:(g + 1) * DK,
                                            c * CH:(c + 1) * CH],
                                    start=True, stop=True,
                                    tile_position=(g * DK, 0),
                                )
                        ecs = []
                        for c in range(NCH):
                            ec = ap2.tile([P, S], BF16, tag=f"e{s2}c{c}",
                                          name=f"e{s2}c{c}")
                            nc.scalar.activation(ec[:], scs[c][:], EXP,
                                                 scale=0.125)
                            ecs.append(ec)
                        exps.append(ecs)
                    return exps

                def pv_phase(a, exps, tail=False):
                    # PV accumulation; ctx_unnorm copied straight out of
                    # psum; the denominator row feeds reciprocal_approx_fast
                    # directly (no gather), results land in r0 at
                    # partition 0 laid out [g0c0|g0c1|g1c0|g1c1].
                    r0 = ap2.tile([1, 2 * S], F32, tag="r0", name="r0",
                                  bufs=1)
                    # both heads' PV matmuls run concurrently on column
                    # strips 0-63 / 64-127 of the array (M=64 each, full
                    # 128-row contraction) -> one psum tile holds both
                    pvs = [psp.tile([P, CH], F32, tag="ps", name="pv",
                                    bufs=4)
                           for _ in range(NCH)]
                    # in the drain, scores' psum slots are idle — park the
                    # long-lived denominator bank there instead of starving
                    # the shared rotation
                    if tail:
                        dn = psp.tile([P, S], F32, tag="sc", name="dn",
                                      bufs=2)
                    else:
                        dn = psp.tile([P, CH], F32, tag="ps", name="dn",
                                      bufs=4)
                    for c in range(NCH):
                        for s2 in range(NT):
                            for g in range(2):
                                nc.tensor.matmul(
                                    pvs[c][g * DK:(g + 1) * DK, :],
                                    lhsT=vv[s2][:, (2 * a + g) * DK:
                                                (2 * a + g + 1) * DK],
                                    rhs=exps[s2][c][:, g * CH:(g + 1) * CH],
                                    start=(s2 == 0),
                                    stop=(s2 == NT - 1),
                                    tile_position=(0, g * DK),
                                )
                    # denominators: colsum of each (g,c) exp block via
                    # ones-matmuls on the four 32-col strips
                    for s2 in range(NT):
                        for i in range(4):
                            g, c = i // 2, i % 2
                            nc.tensor.matmul(
                                dn[32 * i:32 * (i + 1), 0:CH],
                                lhsT=ones32[:],
                                rhs=exps[s2][c][:, g * CH:(g + 1) * CH],
                                start=(s2 == 0),
                                stop=(s2 == NT - 1),
                                tile_position=(0, 32 * i),
                            )
                    for c in range(NCH):
                        cp = nc.scalar.copy if (tail and c == 1) else \
                            nc.vector.tensor_copy
                        cp(ctxT[a][:, c * CH:(c + 1) * CH], pvs[c][:])
                    for i in range(4):
                        g, c = i // 2, i % 2
                        cp = nc.scalar.copy if (tail and c == 1) else \
                            nc.vector.tensor_copy
                        cp(r0[0:1, g * S + c * CH:g * S + (c + 1) * CH],
                           dn[32 * i:32 * i + 1, 0:CH])
                    r0b = ap2.tile([1, 2 * S], BF16, tag="r0b", name="r0b",
                                   bufs=2)
                    if tail:
                        for g in range(2):
                            nc.vector.reciprocal_approx_fast(
                                out=r0[0:1, g * S:(g + 1) * S],
                                in_=r0[0:1, g * S:(g + 1) * S])
                            nc.vector.tensor_copy(
                                r0b[0:1, g * S:(g + 1) * S],
                                r0[0:1, g * S:(g + 1) * S])
                    else:
                        nc.vector.reciprocal_approx_fast(out=r0[:], in_=r0[:])
                        nc.vector.tensor_copy(r0b[:], r0[:])
                    return r0b

                def norm_phase(a, r0b, on_gp=True):
                    # broadcast recip rows across partitions (one per head)
                    # and scale ctxT. Steady state: gpsimd partition
                    # broadcast (off the PE/DVE critical path). Drain: PE
                    # rank-1 matmul broadcast (ones x recip-row -> psum) —
                    # the PE is idle there and the gpsimd path is 1.7us/bcast
                    for g in range(2):
                        if on_gp:
                            rb = ap2.tile([P, S], BF16, tag="rb", name="rb",
                                          bufs=2)
                            nc.gpsimd.partition_broadcast(
                                rb[:], r0b[0:1, g * S:(g + 1) * S])
                            sl = ctxT[a][g * DK:(g + 1) * DK, :]
                            nc.vector.tensor_mul(
                                sl, sl, rb[g * DK:(g + 1) * DK, :])
                        else:
                            if g > 0:
                                continue
                            for c in range(NCH):
                                psb = psp.tile([P, CH], F32, tag="ps",
                                               name="psb", bufs=4)
                                for gg in range(2):
                                    nc.tensor.matmul(
                                        psb[gg * DK:(gg + 1) * DK, :],
                                        lhsT=ones1[0:1, 0:DK],
                                        rhs=r0b[0:1, gg * S + c * CH:
                                                gg * S + (c + 1) * CH],
                                        start=True, stop=True,
                                        tile_position=(0, gg * DK),
                                    )
                                sl = ctxT[a][:, c * CH:(c + 1) * CH]
                                nc.vector.tensor_mul(sl, sl, psb[:])

                # fc partial tiles (bf16) hold ct0-3, later += ct4-6
                fcp = [ap2.tile([P, CH], BF16, tag=f"fp{i}", name=f"fp{i}",
                                bufs=1)
                       for i in range(2 * NT)]
                ones1 = ap2.tile([1, P], BF16, tag="one1", name="ones1",
                                 bufs=1)
                nc.vector.memset(ones1[:], 1.0)
                ones32 = ap2.tile([P, 32], BF16, tag="one32", name="ones32",
                                  bufs=1)
                nc.vector.memset(ones32[:], 1.0)

                def fc_chunk(s1, c, cts, wfs, first):
                    pss = psp.tile([P, CH], F32, tag="ps", name="fcc",
                                   bufs=4)
                    for i, ct in enumerate(cts):
                        nc.tensor.matmul(
                            pss[:],
                            lhsT=ctxT[ct][:, s1 * P:(s1 + 1) * P],
                            rhs=wfs[ct][:, c * CH:(c + 1) * CH],
                            start=(i == 0),
                            stop=(i == len(cts) - 1),
                        )
                    if first:
                        nc.vector.tensor_copy(fcp[s1 * NCH + c][:], pss[:])
                    else:
                        nc.vector.tensor_add(
                            fcp[s1 * NCH + c][:], fcp[s1 * NCH + c][:],
                            pss[:])

                exps_hist = None
                qk_hist = None
                r0_hist = {}
                for a in range(NT):
                    qk_new = qkproj(a)
                    if a >= 2:
                        r0_hist[a - 2] = pv_phase(a - 2, exps_hist)
                    if a >= 1:
                        exps_hist = scores(qk_hist)
                    qk_hist = qk_new
                    if a >= 3:
                        norm_phase(a - 3, r0_hist.pop(a - 3))
                    # fc for ct0-3 interleaves with the last two iterations
                    if a == 6:
                        for s1 in range(4):
                            for c in range(NCH):
                                fc_chunk(s1, c, [0, 1, 2, 3], wf4, True)
                    if a == 7:
                        for s1 in range(4, NT):
                            for c in range(NCH):
                                fc_chunk(s1, c, [0, 1, 2, 3], wf4, True)

                # fc weights for ct 4-7 reuse the q-input slots freed after
                # the last projection
                wf = {}
                for ct in range(4, NT):
                    t_ = ap_.tile([P, S], BF16, tag="xtq", name="wf", bufs=8)
                    nc.sync.dma_start(out=t_[:],
                                      in_=wft[ct * P:(ct + 1) * P, :])
                    wf[ct] = t_

                # drain: sc(7); pv(6)+norm(6); fc ct4-6; pv(7)+norm(7);
                # fc ct7 + final add + store
                exps_last = scores(qk_hist)
                norm_phase(NT - 3, r0_hist.pop(NT - 3), on_gp=False)
                r0 = pv_phase(NT - 2, exps_hist)
                norm_phase(NT - 2, r0, on_gp=False)
                for s1 in range(NT):
                    for c in range(NCH):
                        fc_chunk(s1, c, [4, 5, 6], wf, False)
                r0 = pv_phase(NT - 1, exps_last, tail=True)
                norm_phase(NT - 1, r0, on_gp=False)

                for s1 in range(NT):
                    for c in range(NCH):
                        i = s1 * NCH + c
                        # alternate psum tags: 4 banks for the tail rotation
                        pss = psp.tile([P, CH], F32, tag="ps", name="fc7",
                                       bufs=4)
                        nc.tensor.matmul(
                            pss[:],
                            lhsT=ctxT[NT - 1][:, s1 * P:(s1 + 1) * P],
                            rhs=wf[NT - 1][:, c * CH:(c + 1) * CH],
                            start=True, stop=True,
                        )
                        ob = ap_.tile([P, CH], F32, tag="xtk", name="ob",
                                      bufs=8)
                        if i % 2 == 0:
                            nc.vector.tensor_add(ob[:], fcp[i][:], pss[:])
                        else:
                            # ScalarE evicts psum, GpSimd does the add —
                            # keeps the tail off the DVE-only path
                            st = ap_.tile([P, CH], F32, tag="xtq", name="st",
                                          bufs=8)
                            nc.scalar.copy(st[:], pss[:])
                            nc.gpsimd.tensor_add(ob[:], fcp[i][:], st[:])
                        # out dmas all on the (otherwise idle) sync queue;
                        # chunk only the last two groups so the final
                        # transfer isn't one long single-queue dma
                        if i < 14:
                            nc.sync.dma_start(
                                out=out[s1 * P:(s1 + 1) * P,
                                        c * CH:(c + 1) * CH],
                                in_=ob[:],
                            )
                        else:
                            for hh in range(4):
                                nc.sync.dma_start(
                                    out=out[s1 * P + hh * 32:
                                            s1 * P + (hh + 1) * 32,
                                            c * CH:(c + 1) * CH],
                                    in_=ob[hh * 32:(hh + 1) * 32, :],
                                )

    nc.compile()
    return nc


def run(inputs, trace=False):
    """inputs: dict with Q,K,V [8,1024,1024] and WQ,WK,WV,Wfc [1024,1024].
    Returns (out [8,1024,1024] fp32, exec_time_ns or None)."""
    if "nc" not in _CACHE:
        _CACHE["nc"] = _build()
    nc = _CACHE["nc"]

    import ml_dtypes
    bf16 = ml_dtypes.bfloat16
    f32 = np.float32
    def blockw(w):
        # wb[a, p, d*128+m] = W^T[d*128+p, a*128+m]
        wt = np.asarray(w, dtype=f32).T.astype(bf16)
        return np.ascontiguousarray(
            wt.reshape(8, 128, 8, 128).transpose(2, 1, 0, 3).reshape(
                8, 128, 1024))

    wqb = blockw(inputs["WQ"])
    wkb = blockw(inputs["WK"])
    wvt = np.ascontiguousarray(np.asarray(inputs["WV"], dtype=f32).T.astype(bf16))
    wft = np.ascontiguousarray(np.asarray(inputs["Wfc"], dtype=f32).T.astype(bf16))
    Q = np.asarray(inputs["Q"], dtype=f32)
    K = np.asarray(inputs["K"], dtype=f32)
    V = np.asarray(inputs["V"], dtype=f32)

    in_maps = [
        {
            "xqt": np.ascontiguousarray(Q[b].T.astype(bf16)),
            "xkt": np.ascontiguousarray(K[b].T.astype(bf16)),
            "xvt": np.ascontiguousarray(V[b].T.astype(bf16)),
            "wqb": wqb, "wkb": wkb, "wvt": wvt, "wft": wft,
        }
        for b in range(8)
    ]
    res = run_bass_kernel_spmd(nc, in_maps, core_ids=list(range(8)), trace=trace)
    out = np.stack([res.results[b]["out"] for b in range(8)], axis=0)
    return out.astype(np.float32), res.exec_time_ns


def kernel(**inputs):
    return run(inputs, trace=False)[0]
